# revision 8
# baseline (speedup 1.0000x reference)
"""Trainium2 Bass kernel for the DecoderLSTM problem (8-way model parallel).

kernel(**inputs) takes the FULL inputs and returns (outs, h, c) like the
reference.  Sharding: each core owns a 256-row gate slice (64 h-dims x
{i,f,o,g}), a 125-column vocab slice of W_out, and the full batch on the
SBUF partition dim.  The time loop is unrolled and specialized on the coin
values (teacher-forced vs greedy steps).  Host-side algebraic folds:
  GE = embed_table @ W_ih[:, C:].T   (embedding+input matmul -> row gather)
  G0 = context @ W_ih[:, :C].T + b_ih + b_hh   (loop-invariant)
Per step each core computes its gate slice from the AllGathered h^T, updates
its c/h slice, transposes it, and AllGathers h^T for the next step.  On
greedy steps every core computes replicated full-vocab logits from the same
gathered h^T, so the argmax needs no extra collective.

Each core owns a 256-row gate slice (64 h-dims x {i,f,o,g}), a 125-row vocab
slice of W_out, and the full batch (128) on partitions.  Per step:
  - logits(t-1) for the local vocab slice from the gathered full h(t-1)
  - greedy steps: local argmax candidates -> AllGather -> global argmax
  - GE[idx] row gather (embedding+input matmul folded on host)
  - gates slice = h(t-1) @ whhT_slice + G0_slice + GE[idx]_slice
  - LSTM cell update for the local 64 h-dims, transpose, AllGather h(t).T
"""

import numpy as np

import concourse.bass as bass
import concourse.tile as tile
from concourse import bacc, mybir
from concourse import bass_utils

B, T_FULL, V = 128, 256, 1000
C = H = E = 512
G4 = 4 * H
N_CORES = 8
HS = H // N_CORES    # 64 h-dims per core
GS = 4 * HS          # 256 gate rows per core
VS = V // N_CORES    # 125 vocab rows per core

F32 = mybir.dt.float32
I32 = mybir.dt.int32


def _build_program(greedy, t_steps):
    nc = bacc.Bacc(
        "TRN2", target_bir_lowering=False, debug=False, num_devices=N_CORES
    )

    ge_d = nc.dram_tensor("ge", [V, GS], F32, kind="ExternalInput").ap()
    g0_d = nc.dram_tensor("g0", [B, GS], F32, kind="ExternalInput").ap()
    whhT_d = nc.dram_tensor("whhT", [H, GS], F32, kind="ExternalInput").ap()
    woT_d = nc.dram_tensor("woT", [H, VS], F32, kind="ExternalInput").ap()
    woTf_d = nc.dram_tensor("woTf", [H, V], F32, kind="ExternalInput").ap()
    boutf_d = nc.dram_tensor("boutf", [B, V], F32, kind="ExternalInput").ap()
    bout_d = nc.dram_tensor("bout", [B, VS], F32, kind="ExternalInput").ap()
    ctxT_d = nc.dram_tensor("ctxT", [H, B], F32, kind="ExternalInput").ap()
    tid_d = nc.dram_tensor("tid", [B, t_steps], I32, kind="ExternalInput").ap()
    rev_d = nc.dram_tensor("rev", [B, V], F32, kind="ExternalInput").ap()
    eye_d = nc.dram_tensor("eye", [128, 128], F32, kind="ExternalInput").ap()

    out_lg_d = nc.dram_tensor(
        "out_lg", [B, t_steps, VS], F32, kind="ExternalOutput"
    ).ap()
    out_h_d = nc.dram_tensor("out_h", [B, HS], F32, kind="ExternalOutput").ap()
    out_c_d = nc.dram_tensor("out_c", [B, HS], F32, kind="ExternalOutput").ap()

    RG = [list(range(N_CORES))]

    with tile.TileContext(nc) as tc:
        with (
            tc.tile_pool(name="const", bufs=1) as constp,
            tc.tile_pool(name="state", bufs=1) as statep,
            tc.tile_pool(name="hTp", bufs=2) as hTp,
            tc.tile_pool(name="gep", bufs=3) as gep,
            tc.tile_pool(name="work", bufs=2) as workp,
            tc.tile_pool(name="lgp", bufs=2) as lgp,
            tc.tile_pool(name="dram", bufs=2, space="DRAM") as dramp,
            tc.tile_pool(name="psg", bufs=2, space="PSUM") as psg,
            tc.tile_pool(name="psl", bufs=2, space="PSUM") as psl,
            tc.tile_pool(name="pst", bufs=2, space="PSUM") as pst,
        ):
            # ---- constants ----
            whh_sb = constp.tile([128, 4 * GS], F32)
            for k in range(4):
                nc.sync.dma_start(
                    whh_sb[:, GS * k : GS * (k + 1)],
                    whhT_d[128 * k : 128 * (k + 1), :],
                )
            wo_sb = constp.tile([128, 4 * VS], F32)
            for k in range(4):
                nc.sync.dma_start(
                    wo_sb[:, VS * k : VS * (k + 1)],
                    woT_d[128 * k : 128 * (k + 1), :],
                )
            wof_sb = constp.tile([128, 4 * V], F32)
            for k in range(4):
                nc.sync.dma_start(
                    wof_sb[:, V * k : V * (k + 1)],
                    woTf_d[128 * k : 128 * (k + 1), :],
                )
            boutf_sb = constp.tile([B, V], F32)
            nc.sync.dma_start(boutf_sb[:], boutf_d[:])
            g0_sb = constp.tile([B, GS], F32)
            nc.sync.dma_start(g0_sb[:], g0_d[:])
            rev_sb = constp.tile([B, V], F32)
            nc.sync.dma_start(rev_sb[:], rev_d[:])
            eye_sb = constp.tile([128, 128], F32)
            nc.sync.dma_start(eye_sb[:], eye_d[:])
            bout_sb = constp.tile([B, VS], F32)
            nc.sync.dma_start(bout_sb[:], bout_d[:])
            tid_sb = constp.tile([B, t_steps], I32)
            nc.sync.dma_start(tid_sb[:], tid_d[:])

            c_sb = statep.tile([B, HS], F32)
            nc.vector.memset(c_sb[:], 0.0)

            hT = hTp.tile([128, 4 * B], F32, tag="hT")
            nc.sync.dma_start(
                hT[:].rearrange("p (k b) -> p k b", k=4),
                ctxT_d[:].rearrange("(k p) b -> p k b", p=128),
            )

            lg_prev = None
            h_sb = None

            def emit_logits(tprev, hT_cur):
                lg_ps = psl.tile([B, VS], F32, tag="lg")
                for k in range(4):
                    nc.tensor.matmul(
                        lg_ps[:],
                        hT_cur[:, B * k : B * (k + 1)],
                        wo_sb[:, VS * k : VS * (k + 1)],
                        start=(k == 0),
                        stop=(k == 3),
                    )
                lg_sb = lgp.tile([B, VS], F32, tag="lgs")
                nc.vector.tensor_add(lg_sb[:], lg_ps[:], bout_sb[:])
                nc.sync.dma_start(out_lg_d[:, tprev, :], lg_sb[:])
                return lg_sb

            def emit_logits_full(hT_cur):
                lgf_ps = psl.tile([B, V], F32, tag="lgf", bufs=1)
                for k in range(4):
                    for j0, jn in ((0, 512), (512, 488)):
                        nc.tensor.matmul(
                            lgf_ps[:, j0 : j0 + jn],
                            hT_cur[:, B * k : B * (k + 1)],
                            wof_sb[:, V * k + j0 : V * k + j0 + jn],
                            start=(k == 0),
                            stop=(k == 3),
                        )
                lgf_sb = workp.tile([B, V], F32, tag="lgfs")
                nc.vector.tensor_add(lgf_sb[:], lgf_ps[:], boutf_sb[:])
                return lgf_sb

            for t in range(t_steps):
                # hT currently holds h(t-1)^T (full); logits for step t-1
                if t > 0:
                    lg_prev = emit_logits(t - 1, hT)

                # ---- token index ----
                if greedy[t]:
                    lgf_sb = emit_logits_full(hT)
                    mx = workp.tile([B, 1], F32, tag="mx")
                    nc.vector.tensor_reduce(
                        mx[:], lgf_sb[:], axis=mybir.AxisListType.X,
                        op=mybir.AluOpType.max,
                    )
                    m = workp.tile([B, V], F32, tag="eqm")
                    nc.vector.scalar_tensor_tensor(
                        m[:], lgf_sb[:], mx[:, 0:1], rev_sb[:],
                        op0=mybir.AluOpType.is_equal,
                        op1=mybir.AluOpType.mult,
                    )
                    r = workp.tile([B, 1], F32, tag="r")
                    nc.vector.tensor_reduce(
                        r[:], m[:], axis=mybir.AxisListType.X,
                        op=mybir.AluOpType.max,
                    )
                    idxi = workp.tile([B, 1], I32, tag="idxi")
                    nc.vector.tensor_scalar(
                        idxi[:], r[:], -1.0, float(V),
                        op0=mybir.AluOpType.mult, op1=mybir.AluOpType.add,
                    )
                    off_ap = idxi[:, 0:1]
                else:
                    off_ap = tid_sb[:, t : t + 1]

                ge_t = gep.tile([B, GS], F32, tag="ge")
                nc.gpsimd.indirect_dma_start(
                    out=ge_t[:],
                    out_offset=None,
                    in_=ge_d[:],
                    in_offset=bass.IndirectOffsetOnAxis(ap=off_ap, axis=0),
                )

                # ---- gates slice ----
                gates_ps = psg.tile([B, GS], F32, tag="g")
                for k in range(4):
                    nc.tensor.matmul(
                        gates_ps[:],
                        hT[:, B * k : B * (k + 1)],
                        whh_sb[:, GS * k : GS * (k + 1)],
                        start=(k == 0),
                        stop=(k == 3),
                    )
                pre = workp.tile([B, GS], F32, tag="pre")
                nc.vector.tensor_add(pre[:], gates_ps[:], g0_sb[:])
                nc.vector.tensor_add(pre[:], pre[:], ge_t[:])

                act = workp.tile([B, GS], F32, tag="act")
                nc.scalar.activation(
                    act[:, 0 : 3 * HS], pre[:, 0 : 3 * HS],
                    mybir.ActivationFunctionType.Sigmoid,
                )
                nc.scalar.activation(
                    act[:, 3 * HS : GS], pre[:, 3 * HS : GS],
                    mybir.ActivationFunctionType.Tanh,
                )
                t1 = workp.tile([B, HS], F32, tag="t1")
                nc.vector.tensor_tensor(
                    t1[:], act[:, HS : 2 * HS], c_sb[:], op=mybir.AluOpType.mult
                )
                t2 = workp.tile([B, HS], F32, tag="t2")
                nc.vector.tensor_tensor(
                    t2[:], act[:, 0:HS], act[:, 3 * HS : GS],
                    op=mybir.AluOpType.mult,
                )
                nc.vector.tensor_add(c_sb[:], t1[:], t2[:])
                tc_sb = workp.tile([B, HS], F32, tag="tc")
                nc.scalar.activation(
                    tc_sb[:], c_sb[:], mybir.ActivationFunctionType.Tanh
                )
                h_sb = workp.tile([B, HS], F32, tag="h")
                nc.vector.tensor_tensor(
                    h_sb[:], act[:, 2 * HS : 3 * HS], tc_sb[:],
                    op=mybir.AluOpType.mult,
                )

                # ---- h slice transpose + AllGather ----
                hkT_ps = pst.tile([HS, 128], F32, tag="hkT")
                nc.tensor.transpose(hkT_ps[:], h_sb[:], eye_sb[:])
                hkT_sb = workp.tile([HS, 128], F32, tag="hkTs")
                nc.vector.tensor_copy(hkT_sb[:], hkT_ps[:])
                cc_in_h = dramp.tile([HS, B], F32, tag="ccih")
                nc.sync.dma_start(cc_in_h[:], hkT_sb[:])
                cc_out_h = dramp.tile([H, B], F32, tag="ccoh")
                nc.gpsimd.collective_compute(
                    "AllGather",
                    mybir.AluOpType.bypass,
                    replica_groups=RG,
                    ins=[cc_in_h.opt()],
                    outs=[cc_out_h.opt()],
                )
                hT = hTp.tile([128, 4 * B], F32, tag="hT")
                nc.sync.dma_start(
                    hT[:].rearrange("p (k b) -> p k b", k=4),
                    cc_out_h[:].rearrange("(k p) b -> p k b", p=128),
                )

            emit_logits(t_steps - 1, hT)
            nc.sync.dma_start(out_h_d[:], h_sb[:])
            nc.sync.dma_start(out_c_d[:], c_sb[:])

    nc.compile()
    return nc


def _host_prep(context, target_tensor, coin, embed_table, W_ih, W_hh,
               b_ih, b_hh, W_out, b_out, t_steps):
    f32 = np.float32
    context = np.asarray(context, f32)
    target = np.asarray(target_tensor)
    coin = np.asarray(coin, f32)
    table = np.asarray(embed_table, f32)
    W_ih = np.asarray(W_ih, f32)
    W_hh = np.asarray(W_hh, f32)
    b_ih = np.asarray(b_ih, f32)
    b_hh = np.asarray(b_hh, f32)
    W_out = np.asarray(W_out, f32)
    b_out = np.asarray(b_out, f32)

    # per-core gate rows: [i_k, f_k, o_k, g_k], 64 each (torch order i,f,g,o)
    blocks = {"i": 0, "f": 512, "g": 1024, "o": 1536}
    perm = np.concatenate(
        [
            np.concatenate(
                [
                    np.arange(blocks[b] + HS * k, blocks[b] + HS * (k + 1))
                    for b in ("i", "f", "o", "g")
                ]
            )
            for k in range(N_CORES)
        ]
    )
    Wih_p = W_ih[perm]
    Whh_p = W_hh[perm]
    bias_p = (b_ih.astype(np.float64) + b_hh.astype(np.float64))[perm]

    GE = (table.astype(np.float64) @ Wih_p[:, C:].T.astype(np.float64)).astype(f32)
    G0 = (
        context.astype(np.float64) @ Wih_p[:, :C].T.astype(np.float64) + bias_p
    ).astype(f32)
    whhT = np.ascontiguousarray(Whh_p.T)  # (512, 2048), cols in per-core order
    woT = np.ascontiguousarray(W_out.T)   # (512, 1000)

    tid = np.zeros((B, t_steps), np.int32)
    for t in range(1, t_steps):
        if coin[t] > 0.5:
            tid[:, t] = target[:, t - 1].astype(np.int32)
    greedy = tuple(bool(t > 0 and coin[t] <= 0.5) for t in range(t_steps))

    eye = np.eye(128, dtype=f32)
    ctxT = np.ascontiguousarray(context.T)

    in_maps = []
    for k in range(N_CORES):
        gsl = slice(k * GS, (k + 1) * GS)
        vsl = slice(k * VS, (k + 1) * VS)

        in_maps.append(
            {
                "ge": np.ascontiguousarray(GE[:, gsl]),
                "g0": np.ascontiguousarray(G0[:, gsl]),
                "whhT": np.ascontiguousarray(whhT[:, gsl]),
                "woT": np.ascontiguousarray(woT[:, vsl]),
                "woTf": woT,
                "bout": np.tile(b_out[vsl].reshape(1, VS), (B, 1)),
                "boutf": np.tile(b_out.reshape(1, V), (B, 1)),
                "ctxT": ctxT,
                "tid": tid,
                "rev": np.tile((V - np.arange(V)).astype(f32), (B, 1)),
                "eye": eye,
            }
        )
    return in_maps, greedy


_cache = {}


def _get_program(greedy, t_steps):
    key = (greedy, t_steps)
    if key not in _cache:
        _cache[key] = _build_program(greedy, t_steps)
    return _cache[key]


def run(t_steps=T_FULL, **inputs):
    in_maps, greedy = _host_prep(t_steps=t_steps, **inputs)
    nc = _get_program(greedy, t_steps)
    res = bass_utils.run_bass_kernel_spmd(
        nc, in_maps, core_ids=list(range(N_CORES))
    )
    # outputs: vocab-sliced logits, h/c sliced on hidden dim
    outs = np.concatenate(
        [res.results[k]["out_lg"] for k in range(N_CORES)], axis=2
    )
    h = np.concatenate([res.results[k]["out_h"] for k in range(N_CORES)], 1)
    c = np.concatenate([res.results[k]["out_c"] for k in range(N_CORES)], 1)
    return outs, h, c


def kernel(**inputs):
    return run(T_FULL, **inputs)
